# revision 9
# baseline (speedup 1.0000x reference)
"""Chamfer loss kernel for Trainium2 (8 NeuronCores, SPMD data-parallel over batch).

Problem: pred [8,8192,3], gt [8,8192,3] ->
    scalar = mean_b [ mean_n min_m d(b,n,m) + mean_m min_n d(b,n,m) ]
    d = max(||p-q||^2, 0)

Strategy (one batch element per core):
  - Augmented 5-dim matmul computes the full distance tile directly:
        P~_n = (p0,p1,p2, |p|^2, 1),  Q~_m = (-2q0,-2q1,-2q2, 1, |q|^2)
        dist[n,m] = P~_n . Q~_m
    One K=5 matmul per [128 x 512] output tile (float32r -> full fp32 result
    at 1 cycle/row).
  - Flash-style: PSUM supertiles [128 x 2048], never materialized to HBM.
    DVE keeps a running column-min accumulator [128 x 8192] (tensor_tensor min)
    and running row minima (reduce_min), both straight from PSUM.
  - relu commutes with min, applied after reduction.
  - Per-core output: per-partition row-min sums + colmin sum; host averages.
"""

import os
import sys

import numpy as np

for _p in ("/opt/trn_rl_repo",):
    if os.path.isdir(_p) and _p not in sys.path:
        sys.path.append(_p)

import concourse.bacc as bacc
import concourse.bass as bass
import concourse.mybir as mybir
import concourse.tile as tile
from concourse.bass_utils import run_bass_kernel_spmd
from concourse.masks import make_identity

F32 = mybir.dt.float32
F32R = mybir.dt.float32r
AX = mybir.AxisListType
OP = mybir.AluOpType

BIG = 3.0e38  # "+inf" seed for running minima


def build_chamfer_nc(n: int, m: int, use_f32r: bool = True):
    """Build the per-core chamfer kernel graph.

    Inputs (per core): predT [5, n] f32 (augmented, transposed),
                       gtT   [5, m] f32 (augmented, transposed).
    Output: out [128, 2] f32.
        out[:, 0]  = per-partition sums over n-blocks of relu(row minima)
                     (sum over all of them = sum_n min_m dist)
        out[:, 1]  = per-partition sums of relu(col minima)
    """
    P = 128
    FREE = 2048  # m supertile (4 PSUM banks)
    MMN = 512  # free dim per matmul (1 PSUM bank, fp32)
    assert n % P == 0 and m % FREE == 0
    NB = n // P
    MS = m // FREE
    NMM = FREE // MMN
    assert NMM == 4

    nc = bacc.Bacc("TRN2", target_bir_lowering=False, debug=False)
    mm_dt = F32R if use_f32r else F32
    predT_d = nc.dram_tensor("predT", [5, n], mm_dt, kind="ExternalInput")
    gtT_d = nc.dram_tensor("gtT", [5, m], mm_dt, kind="ExternalInput")
    out_d = nc.dram_tensor("out", [P, 2], F32, kind="ExternalOutput")

    with tile.TileContext(nc) as tc:
        with (
            tc.tile_pool(name="const", bufs=1) as cpool,
            tc.tile_pool(name="psum", bufs=2, space=bass.MemorySpace.PSUM) as ppool,
            tc.tile_pool(name="work", bufs=2) as wpool,
        ):
            # pred/gt replicated at the 4 PE row-groups (base partitions
            # 0/32/64/96) so the 4 matmuls of a supertile run concurrently
            # via tile_position row packing.
            predT = cpool.tile([96 + 5, n], mm_dt)
            gtT = cpool.tile([96 + 5, m], mm_dt)
            for g in range(NMM):
                nc.sync.dma_start(predT[32 * g : 32 * g + 5, :], predT_d[:])
                nc.sync.dma_start(gtT[32 * g : 32 * g + 5, :], gtT_d[:])

            colacc = cpool.tile([P, m], F32)
            nc.gpsimd.memset(colacc[:], BIG)
            rowmins = cpool.tile([P, NB], F32)

            for i in range(NB):
                rowpart = wpool.tile([P, MS], F32, tag="rowpart")
                for J in range(MS):
                    acc = ppool.tile([P, FREE], F32, tag="acc")
                    for j in range(NMM):
                        b = 32 * j
                        lhsT = predT[b : b + 5, i * P : (i + 1) * P]
                        rhs = gtT[
                            b : b + 5, J * FREE + j * MMN : J * FREE + (j + 1) * MMN
                        ]
                        nc.tensor.matmul(
                            acc[:, j * MMN : (j + 1) * MMN],
                            lhsT,
                            rhs,
                            start=True,
                            stop=True,
                            tile_position=(b, 0),
                        )
                    # running row minima (partial per supertile)
                    nc.vector.tensor_reduce(
                        rowpart[:, J : J + 1], acc[:], axis=AX.X, op=OP.min
                    )
                    # running column minima
                    cslice = colacc[:, J * FREE : (J + 1) * FREE]
                    nc.vector.tensor_tensor(cslice, acc[:], cslice, op=OP.min)
                nc.vector.tensor_reduce(
                    rowmins[:, i : i + 1], rowpart[:], axis=AX.X, op=OP.min
                )

            # ---- finalize ----
            # rows: relu then sum -> [P, 1]
            rowrelu = cpool.tile([P, NB], F32)
            rowsum = cpool.tile([P, 1], F32)
            nc.vector.tensor_scalar_max(rowrelu[:], rowmins[:], 0.0)
            nc.vector.tensor_reduce(rowsum[:], rowrelu[:], axis=AX.X, op=OP.add)

            # cols: DVE cannot reduce across partitions. Transpose colacc in
            # 128x128 chunks on the PE, then reduce over the free axis.
            # colminT[mm, k] = min over n-partitions of colacc chunk k.
            ident = cpool.tile([P, P], F32)
            make_identity(nc, ident[:])
            NCH = m // P
            colminT = cpool.tile([P, NCH], F32)
            for k in range(NCH):
                tp = ppool.tile([P, FREE], F32, tag="acc")
                nc.tensor.transpose(
                    tp[:, 0:P], colacc[:, k * P : (k + 1) * P], ident[:]
                )
                nc.vector.tensor_reduce(
                    colminT[:, k : k + 1], tp[:, 0:P], axis=AX.X, op=OP.min
                )
            colrelu = cpool.tile([P, NCH], F32)
            colsum = cpool.tile([P, 1], F32)
            nc.vector.tensor_scalar_max(colrelu[:], colminT[:], 0.0)
            nc.vector.tensor_reduce(colsum[:], colrelu[:], axis=AX.X, op=OP.add)

            out_sb = cpool.tile([P, 2], F32)
            nc.vector.tensor_copy(out_sb[:, 0:1], rowsum[:])
            nc.vector.tensor_copy(out_sb[:, 1:2], colsum[:])
            nc.sync.dma_start(out_d[:], out_sb[:])

    nc.compile()
    return nc


def _augment(pred: np.ndarray, gt: np.ndarray):
    """pred [n,3], gt [m,3] f32 -> predT [5,n], gtT [5,m] f32."""
    n, m = pred.shape[0], gt.shape[0]
    predT = np.empty((5, n), np.float32)
    predT[0:3] = pred.T
    predT[3] = np.sum(pred.astype(np.float64) ** 2, axis=-1).astype(np.float32)
    predT[4] = 1.0
    gtT = np.empty((5, m), np.float32)
    gtT[0:3] = -2.0 * gt.T
    gtT[3] = 1.0
    gtT[4] = np.sum(gt.astype(np.float64) ** 2, axis=-1).astype(np.float32)
    return predT, gtT


_NC_CACHE = {}


def _get_nc(n, m, use_f32r=True):
    key = (n, m, use_f32r)
    if key not in _NC_CACHE:
        _NC_CACHE[key] = build_chamfer_nc(n, m, use_f32r)
    return _NC_CACHE[key]


def run_chamfer(pred: np.ndarray, gt: np.ndarray, use_f32r: bool = True, **kw):
    """pred [B,N,3], gt [B,M,3] -> (scalar, BassKernelResults)."""
    B, N, _ = pred.shape
    M = gt.shape[1]
    assert B <= 8
    nc = _get_nc(N, M, use_f32r)
    in_maps = []
    for b in range(B):
        predT, gtT = _augment(
            np.ascontiguousarray(pred[b], np.float32),
            np.ascontiguousarray(gt[b], np.float32),
        )
        in_maps.append({"predT": predT, "gtT": gtT})
    res = run_bass_kernel_spmd(nc, in_maps, core_ids=list(range(B)), **kw)
    vals = []
    for r in res.results:
        o = r["out"]
        p2q = float(o[:, 0].sum()) / N
        q2p = float(o[:, 1].sum()) / M
        vals.append(p2q + q2p)
    return np.float32(np.mean(vals)), res


def kernel(pred: np.ndarray, gt: np.ndarray) -> np.ndarray:
    val, _ = run_chamfer(np.asarray(pred), np.asarray(gt))
    return np.array(val, dtype=np.float32)


# revision 13
# speedup vs baseline: 1.4234x; 1.4234x over previous
"""Chamfer loss kernel for Trainium2 (8 NeuronCores, SPMD data-parallel over batch).

Problem: pred [8,8192,3], gt [8,8192,3] ->
    scalar = mean_b [ mean_n min_m d(b,n,m) + mean_m min_n d(b,n,m) ]
    d = max(||p-q||^2, 0)

Strategy (one batch element per core):
  - Augmented 5-dim matmul computes the full distance tile directly:
        P~_n = (p0,p1,p2, |p|^2, 1),  Q~_m = (-2q0,-2q1,-2q2, 1, |q|^2)
        dist[n,m] = P~_n . Q~_m
    float32r -> full-fp32-accurate result at ~1 cycle/row. The 4 matmuls of a
    [128 x 2048] PSUM supertile are packed into the 4 PE row groups
    (tile_position) so they run concurrently.
  - Flash-style: distances only ever exist in PSUM. A single custom DVE op
    (CHAMFER_MIN_MIN) per supertile does BOTH reductions in one pass:
        out      = min(tile, colacc)          (running column minima)
        accum    = min(seed, min_free(tile))  (running row minima)
    The stock Spec DSL can only fold the body (which would contaminate row
    minima with other row-blocks' values via colacc); we hand-edit the lowered
    uop so the accumulator ALU reads the raw Src0 delay lane instead of the
    body ALU output.
  - relu commutes with min -> applied after reduction.
  - Tail: col minima across partitions via PE transpose chunks + DVE reduce.
  - Per-core output: per-partition row/col relu'd min sums; host averages.
"""

import os
import sys

import numpy as np

for _p in ("/opt/trn_rl_repo",):
    if os.path.isdir(_p) and _p not in sys.path:
        sys.path.append(_p)

import concourse.bacc as bacc
import concourse.bass as bass
import concourse.mybir as mybir
import concourse.tile as tile
from concourse.bass_utils import run_bass_kernel_spmd
from concourse.masks import make_identity

F32 = mybir.dt.float32
F32R = mybir.dt.float32r
AX = mybir.AxisListType
OP = mybir.AluOpType

BIG = 3.0e38  # "+inf" seed for running minima


# ---------------------------------------------------------------------------
# Custom DVE op: out = min(in0, in1); accum_out = min(s0, min_free(in0))
# ---------------------------------------------------------------------------
def _register_chamfer_op():
    from concourse import dve_ops
    from concourse.dve_spec import Spec, Src0, Src1, minn, lower, AluOp, C0
    from concourse.dve_uop import AluInp, DveOpSpec

    name = "CHAMFER_MIN_MIN"
    if name in dve_ops._SUB_OPCODE_FOR_NAME:
        for op in dve_ops.OPS:
            if op.name == name:
                return op

    def _ref(in0, in1, c0, c1, c2):
        out = np.minimum(in0, in1)
        accum = np.minimum(in0.min(axis=-1, keepdims=True), c0)
        return out, accum

    spec = Spec(
        body=minn(Src0, Src1),
        accum=AluOp.MIN,
        accum_init=C0,
        reference=_ref,
    )
    uops = lower(spec, ver="v3")
    # uops[0] = seed state (accumulator <- C0), uops[1] = steady state with
    # blk1 = MIN(CURR_ALU_OUT, PREV_ALU_OUT) i.e. accum folds the body.
    # Repoint the accumulator's input to the raw Src0 carried on delay lane 0
    # out of blk0 (same pipeline tick as the body value) so the row-min is
    # computed from the tile alone, uncontaminated by colacc.
    st = uops[1].datapath_config[1]
    assert st.op == AluOp.MIN and st.alu_src0 == AluInp.CURR_ALU_OUT, (
        "dve_spec lowering layout changed; revisit CHAMFER_MIN_MIN uop edit"
    )
    # src0 = CURR_ALU_OUT is the accumulator feedback (blk1's own out flop);
    # src1 = PREV_ALU_OUT is the body value from blk0. Swap src1 to the raw
    # Src0 riding blk0's delay lane 0 (same pipeline tick as the body value).
    assert st.alu_src1 == AluInp.PREV_ALU_OUT
    st.alu_src1 = AluInp.PREV_DELAY_0

    opcode = max(dve_ops._SUB_OPCODE_FOR_NAME.values()) + 1
    assert opcode < 0x20

    class _HandEditedOp:
        def __init__(self):
            self.name = name
            self.spec = spec
            self.subdim = False
            self.perf_en = {}
            self._compiled = {}

        def compile(self, ver):
            assert ver == "v3", "CHAMFER_MIN_MIN is TRN2-only"
            if ver not in self._compiled:
                self._compiled[ver] = DveOpSpec(
                    name=self.name, opcode=opcode, uops=uops, rd1_en=True
                )
            return self._compiled[ver]

    op = _HandEditedOp()
    dve_ops.OPS.append(op)
    dve_ops._SUB_OPCODE_FOR_NAME[name] = opcode
    return op


CHAMFER_OP = _register_chamfer_op()


def build_chamfer_nc(n: int, m: int, use_f32r: bool = True):
    """Build the per-core chamfer kernel graph.

    Inputs (per core): predT [5, n] f32 (augmented, transposed),
                       gtT   [5, m] f32 (augmented, transposed).
    Output: out [128, 2] f32.
        out[:, 0]  = per-partition sums over n-blocks of relu(row minima)
        out[:, 1]  = per-partition sums of relu(col minima)
    """
    P = 128
    FREE = 2048  # m supertile (4 PSUM banks)
    MMN = 512  # free dim per matmul (1 PSUM bank, fp32)
    assert n % P == 0 and m % FREE == 0
    NB = n // P
    MS = m // FREE
    NMM = FREE // MMN
    assert NMM == 4

    nc = bacc.Bacc("TRN2", target_bir_lowering=False, debug=False)
    mm_dt = F32R if use_f32r else F32
    predT_d = nc.dram_tensor("predT", [5, n], mm_dt, kind="ExternalInput")
    gtT_d = nc.dram_tensor("gtT", [5, m], mm_dt, kind="ExternalInput")
    out_d = nc.dram_tensor("out", [P, 2], F32, kind="ExternalOutput")

    with tile.TileContext(nc) as tc:
        with (
            tc.tile_pool(name="const", bufs=1) as cpool,
            tc.tile_pool(name="psum", bufs=2, space=bass.MemorySpace.PSUM) as ppool,
        ):
            # pred/gt replicated at the 4 PE row-groups (base partitions
            # 0/32/64/96) so the 4 matmuls of a supertile run concurrently
            # via tile_position row packing.
            predT = cpool.tile([96 + 5, n], mm_dt)
            gtT = cpool.tile([96 + 5, m], mm_dt)
            for g in range(NMM):
                nc.sync.dma_start(predT[32 * g : 32 * g + 5, :], predT_d[:])
                nc.sync.dma_start(gtT[32 * g : 32 * g + 5, :], gtT_d[:])

            colacc = cpool.tile([P, m], F32)
            nc.gpsimd.memset(colacc[:], BIG)
            rowmins = cpool.tile([P, NB], F32)
            nc.vector.memset(rowmins[:], BIG)

            for i in range(NB):
                for J in range(MS):
                    acc = ppool.tile([P, FREE], F32, tag="acc")
                    for j in range(NMM):
                        b = 32 * j
                        lhsT = predT[b : b + 5, i * P : (i + 1) * P]
                        rhs = gtT[
                            b : b + 5, J * FREE + j * MMN : J * FREE + (j + 1) * MMN
                        ]
                        nc.tensor.matmul(
                            acc[:, j * MMN : (j + 1) * MMN],
                            lhsT,
                            rhs,
                            start=True,
                            stop=True,
                            tile_position=(b, 0),
                        )
                    # fused: colacc slice gets elementwise min; rowmins[:, i]
                    # accumulates the tile's row minima (seeded by itself).
                    cslice = colacc[:, J * FREE : (J + 1) * FREE]
                    nc.vector._custom_dve(
                        CHAMFER_OP,
                        out=cslice,
                        accum_out=rowmins[:, i : i + 1],
                        in0=acc[:],
                        in1=cslice,
                        s0=rowmins[:, i : i + 1],
                    )

            # ---- finalize ----
            # rows: relu then sum -> [P, 1]
            rowrelu = cpool.tile([P, NB], F32)
            rowsum = cpool.tile([P, 1], F32)
            nc.vector.tensor_scalar_max(rowrelu[:], rowmins[:], 0.0)
            nc.vector.tensor_reduce(rowsum[:], rowrelu[:], axis=AX.X, op=OP.add)

            # cols: DVE cannot reduce across partitions. Transpose colacc in
            # 128x128 chunks on the PE, then reduce over the free axis.
            ident = cpool.tile([P, P], F32)
            make_identity(nc, ident[:])
            NCH = m // P
            colminT = cpool.tile([P, NCH], F32)
            for k in range(NCH):
                tp = ppool.tile([P, FREE], F32, tag="acc")
                nc.tensor.transpose(
                    tp[:, 0:P], colacc[:, k * P : (k + 1) * P], ident[:]
                )
                nc.vector.tensor_reduce(
                    colminT[:, k : k + 1], tp[:, 0:P], axis=AX.X, op=OP.min
                )
            colrelu = cpool.tile([P, NCH], F32)
            colsum = cpool.tile([P, 1], F32)
            nc.vector.tensor_scalar_max(colrelu[:], colminT[:], 0.0)
            nc.vector.tensor_reduce(colsum[:], colrelu[:], axis=AX.X, op=OP.add)

            out_sb = cpool.tile([P, 2], F32)
            nc.vector.tensor_copy(out_sb[:, 0:1], rowsum[:])
            nc.vector.tensor_copy(out_sb[:, 1:2], colsum[:])
            nc.sync.dma_start(out_d[:], out_sb[:])

    nc.compile()
    return nc


def _augment(pred: np.ndarray, gt: np.ndarray):
    """pred [n,3], gt [m,3] f32 -> predT [5,n], gtT [5,m] f32."""
    n, m = pred.shape[0], gt.shape[0]
    predT = np.empty((5, n), np.float32)
    predT[0:3] = pred.T
    predT[3] = np.sum(pred.astype(np.float64) ** 2, axis=-1).astype(np.float32)
    predT[4] = 1.0
    gtT = np.empty((5, m), np.float32)
    gtT[0:3] = -2.0 * gt.T
    gtT[3] = 1.0
    gtT[4] = np.sum(gt.astype(np.float64) ** 2, axis=-1).astype(np.float32)
    return predT, gtT


_NC_CACHE = {}


def _get_nc(n, m, use_f32r=True):
    key = (n, m, use_f32r)
    if key not in _NC_CACHE:
        _NC_CACHE[key] = build_chamfer_nc(n, m, use_f32r)
    return _NC_CACHE[key]


def run_chamfer(pred: np.ndarray, gt: np.ndarray, use_f32r: bool = True, **kw):
    """pred [B,N,3], gt [B,M,3] -> (scalar, BassKernelResults)."""
    B, N, _ = pred.shape
    M = gt.shape[1]
    assert B <= 8
    nc = _get_nc(N, M, use_f32r)
    in_maps = []
    for b in range(B):
        predT, gtT = _augment(
            np.ascontiguousarray(pred[b], np.float32),
            np.ascontiguousarray(gt[b], np.float32),
        )
        in_maps.append({"predT": predT, "gtT": gtT})
    res = run_bass_kernel_spmd(nc, in_maps, core_ids=list(range(B)), **kw)
    vals = []
    for r in res.results:
        o = r["out"]
        p2q = float(o[:, 0].sum()) / N
        q2p = float(o[:, 1].sum()) / M
        vals.append(p2q + q2p)
    return np.float32(np.mean(vals)), res


def kernel(pred: np.ndarray, gt: np.ndarray) -> np.ndarray:
    val, _ = run_chamfer(np.asarray(pred), np.asarray(gt))
    return np.array(val, dtype=np.float32)


# revision 14
# speedup vs baseline: 1.7440x; 1.2252x over previous
"""Chamfer loss kernel for Trainium2 (8 NeuronCores, SPMD data-parallel over batch).

Problem: pred [8,8192,3], gt [8,8192,3] ->
    scalar = mean_b [ mean_n min_m d(b,n,m) + mean_m min_n d(b,n,m) ]
    d = max(||p-q||^2, 0)

Strategy (one batch element per core):
  - Augmented 5-dim matmul computes the full distance tile directly:
        P~_n = (p0,p1,p2, |p|^2, 1),  Q~_m = (-2q0,-2q1,-2q2, 1, |q|^2)
        dist[n,m] = P~_n . Q~_m
    float32r -> full-fp32-accurate result at ~1 cycle/row. The 4 matmuls of a
    [128 x 2048] PSUM supertile are packed into the 4 PE row groups
    (tile_position) so they run concurrently.
  - Flash-style: distances only ever exist in PSUM. A single custom DVE op
    (CHAMFER_MIN_MIN) per supertile does BOTH reductions in one pass:
        out      = min(tile, colacc)          (running column minima)
        accum    = min(seed, min_free(tile))  (running row minima)
    The stock Spec DSL can only fold the body (which would contaminate row
    minima with other row-blocks' values via colacc); we hand-edit the lowered
    uop so the accumulator ALU reads the raw Src0 delay lane instead of the
    body ALU output.
  - relu commutes with min -> applied after reduction.
  - Tail: col minima across partitions via PE transpose chunks + DVE reduce.
  - Per-core output: per-partition row/col relu'd min sums; host averages.
"""

import os
import sys

import numpy as np

for _p in ("/opt/trn_rl_repo",):
    if os.path.isdir(_p) and _p not in sys.path:
        sys.path.append(_p)

import concourse.bacc as bacc
import concourse.bass as bass
import concourse.mybir as mybir
import concourse.tile as tile
from concourse.bass_utils import run_bass_kernel_spmd
from concourse.masks import make_identity

F32 = mybir.dt.float32
F32R = mybir.dt.float32r
AX = mybir.AxisListType
OP = mybir.AluOpType

BIG = 3.0e38  # "+inf" seed for running minima


# ---------------------------------------------------------------------------
# Custom DVE op: out = min(in0, in1); accum_out = min(s0, min_free(in0))
# ---------------------------------------------------------------------------
def _register_chamfer_op():
    from concourse import dve_ops
    from concourse.dve_spec import Spec, Src0, Src1, minn, lower, AluOp, C0
    from concourse.dve_uop import AluInp, DveOpSpec

    name = "CHAMFER_MIN_MIN"
    if name in dve_ops._SUB_OPCODE_FOR_NAME:
        for op in dve_ops.OPS:
            if op.name == name:
                return op

    def _ref(in0, in1, c0, c1, c2):
        out = np.minimum(in0, in1)
        accum = np.minimum(in0.min(axis=-1, keepdims=True), c0)
        return out, accum

    spec = Spec(
        body=minn(Src0, Src1),
        accum=AluOp.MIN,
        accum_init=C0,
        reference=_ref,
    )
    uops = lower(spec, ver="v3")
    # uops[0] = seed state (accumulator <- C0), uops[1] = steady state with
    # blk1 = MIN(CURR_ALU_OUT, PREV_ALU_OUT) i.e. accum folds the body.
    # Repoint the accumulator's input to the raw Src0 carried on delay lane 0
    # out of blk0 (same pipeline tick as the body value) so the row-min is
    # computed from the tile alone, uncontaminated by colacc.
    st = uops[1].datapath_config[1]
    assert st.op == AluOp.MIN and st.alu_src0 == AluInp.CURR_ALU_OUT, (
        "dve_spec lowering layout changed; revisit CHAMFER_MIN_MIN uop edit"
    )
    # src0 = CURR_ALU_OUT is the accumulator feedback (blk1's own out flop);
    # src1 = PREV_ALU_OUT is the body value from blk0. Swap src1 to the raw
    # Src0 riding blk0's delay lane 0 (same pipeline tick as the body value).
    assert st.alu_src1 == AluInp.PREV_ALU_OUT
    st.alu_src1 = AluInp.PREV_DELAY_0

    opcode = max(dve_ops._SUB_OPCODE_FOR_NAME.values()) + 1
    assert opcode < 0x20

    class _HandEditedOp:
        def __init__(self):
            self.name = name
            self.spec = spec
            self.subdim = False
            self.perf_en = {}
            self._compiled = {}

        def compile(self, ver):
            assert ver == "v3", "CHAMFER_MIN_MIN is TRN2-only"
            if ver not in self._compiled:
                self._compiled[ver] = DveOpSpec(
                    name=self.name, opcode=opcode, uops=uops, rd1_en=True
                )
            return self._compiled[ver]

    op = _HandEditedOp()
    dve_ops.OPS.append(op)
    dve_ops._SUB_OPCODE_FOR_NAME[name] = opcode
    return op


CHAMFER_OP = _register_chamfer_op()


def build_chamfer_nc(n: int, m: int, use_f32r: bool = True):
    """Build the per-core chamfer kernel graph.

    Inputs (per core): predT [5, n] f32 (augmented, transposed),
                       gtT   [5, m] f32 (augmented, transposed).
    Output: out [128, 2] f32.
        out[:, 0]  = per-partition sums over n-blocks of relu(row minima)
        out[:, 1]  = per-partition sums of relu(col minima)
    """
    P = 128
    FREE = 2048  # m supertile (4 PSUM banks)
    MMN = 512  # free dim per matmul (1 PSUM bank, fp32)
    assert n % P == 0 and m % FREE == 0
    NB = n // P
    MS = m // FREE
    NMM = FREE // MMN
    assert NMM == 4

    nc = bacc.Bacc("TRN2", target_bir_lowering=False, debug=False)
    mm_dt = F32R if use_f32r else F32
    predT_d = nc.dram_tensor("predT", [5, n], mm_dt, kind="ExternalInput")
    gtT_d = nc.dram_tensor("gtT", [5, m], mm_dt, kind="ExternalInput")
    out_d = nc.dram_tensor("out", [P, 2], F32, kind="ExternalOutput")

    with tile.TileContext(nc) as tc:
        with (
            tc.tile_pool(name="const", bufs=1) as cpool,
            tc.tile_pool(name="psum", bufs=2, space=bass.MemorySpace.PSUM) as ppool,
        ):
            # pred/gt replicated at the 4 PE row-groups (base partitions
            # 0/32/64/96) so the 4 matmuls of a supertile run concurrently
            # via tile_position row packing.
            predT = cpool.tile([96 + 5, n], mm_dt)
            gtT = cpool.tile([96 + 5, m], mm_dt)
            for g in range(NMM):
                nc.sync.dma_start(predT[32 * g : 32 * g + 5, :], predT_d[:])
                nc.sync.dma_start(gtT[32 * g : 32 * g + 5, :], gtT_d[:])

            colacc = cpool.tile([P, m], F32)
            nc.gpsimd.memset(colacc[:], BIG)
            rowmins = cpool.tile([P, NB], F32)
            nc.vector.memset(rowmins[:], BIG)

            for i in range(NB):
                for J in range(MS):
                    acc = ppool.tile([P, FREE], F32, tag="acc")
                    for j in range(NMM):
                        b = 32 * j
                        lhsT = predT[b : b + 5, i * P : (i + 1) * P]
                        rhs = gtT[
                            b : b + 5, J * FREE + j * MMN : J * FREE + (j + 1) * MMN
                        ]
                        nc.tensor.matmul(
                            acc[:, j * MMN : (j + 1) * MMN],
                            lhsT,
                            rhs,
                            start=True,
                            stop=True,
                            tile_position=(b, 0),
                        )
                    # fused: colacc slice gets elementwise min; rowmins[:, i]
                    # accumulates the tile's row minima (seeded by itself).
                    cslice = colacc[:, J * FREE : (J + 1) * FREE]
                    nc.vector._custom_dve(
                        CHAMFER_OP,
                        out=cslice,
                        accum_out=rowmins[:, i : i + 1],
                        in0=acc[:],
                        in1=cslice,
                        s0=rowmins[:, i : i + 1],
                    )

            # ---- finalize ----
            # rows: relu then sum -> [P, 1]
            rowrelu = cpool.tile([P, NB], F32)
            rowsum = cpool.tile([P, 1], F32)
            nc.vector.tensor_scalar_max(rowrelu[:], rowmins[:], 0.0)
            nc.vector.tensor_reduce(rowsum[:], rowrelu[:], axis=AX.X, op=OP.add)

            # cols: DVE cannot reduce across partitions. Transpose colacc in
            # 128x128 chunks on the PE (16 chunks per PSUM tile), then one 3D
            # reduce [P, 16, P] -> [P, 16] per tile.
            ident = cpool.tile([P, P], F32)
            make_identity(nc, ident[:])
            NCH = m // P
            CPT = FREE // P  # transposed chunks per psum tile
            colminT = cpool.tile([P, NCH], F32)
            for t in range(NCH // CPT):
                tp = ppool.tile([P, FREE], F32, tag="acc")
                for k in range(CPT):
                    kk = t * CPT + k
                    nc.tensor.transpose(
                        tp[:, k * P : (k + 1) * P],
                        colacc[:, kk * P : (kk + 1) * P],
                        ident[:],
                    )
                nc.vector.tensor_reduce(
                    colminT[:, t * CPT : (t + 1) * CPT],
                    tp[:].rearrange("p (k c) -> p k c", c=P),
                    axis=AX.X,
                    op=OP.min,
                )
            colrelu = cpool.tile([P, NCH], F32)
            colsum = cpool.tile([P, 1], F32)
            nc.vector.tensor_scalar_max(colrelu[:], colminT[:], 0.0)
            nc.vector.tensor_reduce(colsum[:], colrelu[:], axis=AX.X, op=OP.add)

            out_sb = cpool.tile([P, 2], F32)
            nc.vector.tensor_copy(out_sb[:, 0:1], rowsum[:])
            nc.vector.tensor_copy(out_sb[:, 1:2], colsum[:])
            nc.sync.dma_start(out_d[:], out_sb[:])

    nc.compile()
    return nc


def _augment(pred: np.ndarray, gt: np.ndarray):
    """pred [n,3], gt [m,3] f32 -> predT [5,n], gtT [5,m] f32."""
    n, m = pred.shape[0], gt.shape[0]
    predT = np.empty((5, n), np.float32)
    predT[0:3] = pred.T
    predT[3] = np.sum(pred.astype(np.float64) ** 2, axis=-1).astype(np.float32)
    predT[4] = 1.0
    gtT = np.empty((5, m), np.float32)
    gtT[0:3] = -2.0 * gt.T
    gtT[3] = 1.0
    gtT[4] = np.sum(gt.astype(np.float64) ** 2, axis=-1).astype(np.float32)
    return predT, gtT


_NC_CACHE = {}


def _get_nc(n, m, use_f32r=True):
    key = (n, m, use_f32r)
    if key not in _NC_CACHE:
        _NC_CACHE[key] = build_chamfer_nc(n, m, use_f32r)
    return _NC_CACHE[key]


def run_chamfer(pred: np.ndarray, gt: np.ndarray, use_f32r: bool = True, **kw):
    """pred [B,N,3], gt [B,M,3] -> (scalar, BassKernelResults)."""
    B, N, _ = pred.shape
    M = gt.shape[1]
    assert B <= 8
    nc = _get_nc(N, M, use_f32r)
    in_maps = []
    for b in range(B):
        predT, gtT = _augment(
            np.ascontiguousarray(pred[b], np.float32),
            np.ascontiguousarray(gt[b], np.float32),
        )
        in_maps.append({"predT": predT, "gtT": gtT})
    res = run_bass_kernel_spmd(nc, in_maps, core_ids=list(range(B)), **kw)
    vals = []
    for r in res.results:
        o = r["out"]
        p2q = float(o[:, 0].sum()) / N
        q2p = float(o[:, 1].sum()) / M
        vals.append(p2q + q2p)
    return np.float32(np.mean(vals)), res


def kernel(pred: np.ndarray, gt: np.ndarray) -> np.ndarray:
    val, _ = run_chamfer(np.asarray(pred), np.asarray(gt))
    return np.array(val, dtype=np.float32)


# revision 16
# speedup vs baseline: 1.9280x; 1.1055x over previous
"""Chamfer loss kernel for Trainium2 (8 NeuronCores, SPMD data-parallel over batch).

Problem: pred [8,8192,3], gt [8,8192,3] ->
    scalar = mean_b [ mean_n min_m d(b,n,m) + mean_m min_n d(b,n,m) ]
    d = max(||p-q||^2, 0)

Strategy (one batch element per core):
  - Augmented 5-dim matmul computes the full distance tile directly:
        P~_n = (p0,p1,p2, |p|^2, 1),  Q~_m = (-2q0,-2q1,-2q2, 1, |q|^2)
        dist[n,m] = P~_n . Q~_m
    float32r -> full-fp32-accurate result at ~1 cycle/row. The 4 matmuls of a
    [128 x 2048] PSUM supertile are packed into the 4 PE row groups
    (tile_position) so they run concurrently.
  - Flash-style: distances only ever exist in PSUM. A single custom DVE op
    (CHAMFER_MIN_MIN) per supertile does BOTH reductions in one pass:
        out      = min(tile, colacc)          (running column minima)
        accum    = min(seed, min_free(tile))  (running row minima)
    The stock Spec DSL can only fold the body (which would contaminate row
    minima with other row-blocks' values via colacc); we hand-edit the lowered
    uop so the accumulator ALU reads the raw Src0 delay lane instead of the
    body ALU output.
  - relu commutes with min -> applied after reduction.
  - Tail: col minima across partitions via PE transpose chunks + DVE reduce.
  - Per-core output: per-partition row/col relu'd min sums; host averages.
"""

import os
import sys

import numpy as np

for _p in ("/opt/trn_rl_repo",):
    if os.path.isdir(_p) and _p not in sys.path:
        sys.path.append(_p)

import concourse.bacc as bacc
import concourse.bass as bass
import concourse.mybir as mybir
import concourse.tile as tile
from concourse.bass_utils import run_bass_kernel_spmd
from concourse.masks import make_identity

F32 = mybir.dt.float32
F32R = mybir.dt.float32r
AX = mybir.AxisListType
OP = mybir.AluOpType

BIG = 3.0e38  # "+inf" seed for running minima


# ---------------------------------------------------------------------------
# Custom DVE op: out = min(in0, in1); accum_out = min(s0, min_free(in0))
# ---------------------------------------------------------------------------
def _register_chamfer_ops():
    import copy

    from concourse import dve_ops
    from concourse.dve_spec import Spec, Src0, Src1, minn, lower, AluOp, C0
    from concourse.dve_uop import AluInp, DveOpSpec

    if "CHAMFER_MIN_MIN" in dve_ops._SUB_OPCODE_FOR_NAME:
        found = {op.name: op for op in dve_ops.OPS}
        return found["CHAMFER_MIN_MIN"], found["CHAMFER_MIN_MIN_CHAIN"]

    def _ref(in0, in1, c0, c1, c2):
        out = np.minimum(in0, in1)
        accum = np.minimum(in0.min(axis=-1, keepdims=True), c0)
        return out, accum

    spec = Spec(
        body=minn(Src0, Src1),
        accum=AluOp.MIN,
        accum_init=C0,
        reference=_ref,
    )
    uops = lower(spec, ver="v3")
    # uops[0] = seed state (accumulator <- C0), uops[1] = steady state with
    # blk1 = MIN(src0=CURR_ALU_OUT, src1=PREV_ALU_OUT): src0 is the
    # accumulator feedback (blk1's own out flop), src1 the body value from
    # blk0. Repoint src1 to the raw Src0 riding blk0's delay lane 0 (same
    # pipeline tick as the body value) so the row-min accumulates the tile
    # alone, uncontaminated by colacc.
    st = uops[1].datapath_config[1]
    assert st.op == AluOp.MIN and st.alu_src0 == AluInp.CURR_ALU_OUT, (
        "dve_spec lowering layout changed; revisit CHAMFER_MIN_MIN uop edit"
    )
    assert st.alu_src1 == AluInp.PREV_ALU_OUT
    st.alu_src1 = AluInp.PREV_DELAY_0

    # Chain variant: uop0's accumulator stage holds the register value left by
    # the previous CHAMFER op (BYPASS of its own flop) instead of reseeding
    # from C0. Lets one row-block's 4 supertile ops share one accumulator with
    # a single readout on the last op.
    uops_chain = copy.deepcopy(uops)
    s0 = uops_chain[0].datapath_config[1]
    assert s0.op == AluOp.BYPASS and s0.alu_src0 == AluInp.PREV_DELAY_2
    s0.alu_src0 = AluInp.CURR_ALU_OUT
    s0.alu_src1 = AluInp.CURR_ALU_OUT

    base = max(dve_ops._SUB_OPCODE_FOR_NAME.values())
    assert base + 2 < 0x20

    class _HandEditedOp:
        def __init__(self, name, the_uops, opcode):
            self.name = name
            self.spec = spec
            self.subdim = False
            self.perf_en = {}
            self._opcode = opcode
            self._uops = the_uops
            self._compiled = {}

        def compile(self, ver):
            assert ver == "v3", "CHAMFER ops are TRN2-only"
            if ver not in self._compiled:
                self._compiled[ver] = DveOpSpec(
                    name=self.name, opcode=self._opcode, uops=self._uops, rd1_en=True
                )
            return self._compiled[ver]

    op_seed = _HandEditedOp("CHAMFER_MIN_MIN", uops, base + 1)
    op_chain = _HandEditedOp("CHAMFER_MIN_MIN_CHAIN", uops_chain, base + 2)
    for op in (op_seed, op_chain):
        dve_ops.OPS.append(op)
        dve_ops._SUB_OPCODE_FOR_NAME[op.name] = op._opcode
    return op_seed, op_chain


CHAMFER_OP, CHAMFER_OP_CHAIN = _register_chamfer_ops()
USE_CHAIN = True


def build_chamfer_nc(n: int, m: int, use_f32r: bool = True):
    """Build the per-core chamfer kernel graph.

    Inputs (per core): predT [5, n] f32 (augmented, transposed),
                       gtT   [5, m] f32 (augmented, transposed).
    Output: out [128, 2] f32.
        out[:, 0]  = per-partition sums over n-blocks of relu(row minima)
        out[:, 1]  = per-partition sums of relu(col minima)
    """
    P = 128
    FREE = 2048  # m supertile (4 PSUM banks)
    MMN = 512  # free dim per matmul (1 PSUM bank, fp32)
    assert n % P == 0 and m % FREE == 0
    NB = n // P
    MS = m // FREE
    NMM = FREE // MMN
    assert NMM == 4

    nc = bacc.Bacc("TRN2", target_bir_lowering=False, debug=False)
    mm_dt = F32R if use_f32r else F32
    predT_d = nc.dram_tensor("predT", [5, n], mm_dt, kind="ExternalInput")
    gtT_d = nc.dram_tensor("gtT", [5, m], mm_dt, kind="ExternalInput")
    out_d = nc.dram_tensor("out", [P, 2], F32, kind="ExternalOutput")

    with tile.TileContext(nc) as tc:
        with (
            tc.tile_pool(name="const", bufs=1) as cpool,
            tc.tile_pool(name="psum", bufs=2, space=bass.MemorySpace.PSUM) as ppool,
        ):
            # pred/gt replicated at the 4 PE row-groups (base partitions
            # 0/32/64/96) so the 4 matmuls of a supertile run concurrently
            # via tile_position row packing.
            predT = cpool.tile([96 + 5, n], mm_dt)
            gtT = cpool.tile([96 + 5, m], mm_dt)
            for g in range(NMM):
                nc.sync.dma_start(predT[32 * g : 32 * g + 5, :], predT_d[:])
                nc.sync.dma_start(gtT[32 * g : 32 * g + 5, :], gtT_d[:])

            colacc = cpool.tile([P, m], F32)
            nc.gpsimd.memset(colacc[:], BIG)
            rowmins = cpool.tile([P, NB], F32)
            nc.vector.memset(rowmins[:], BIG)

            prev_inst = None
            for i in range(NB):
                for J in range(MS):
                    acc = ppool.tile([P, FREE], F32, tag="acc")
                    for j in range(NMM):
                        b = 32 * j
                        lhsT = predT[b : b + 5, i * P : (i + 1) * P]
                        rhs = gtT[
                            b : b + 5, J * FREE + j * MMN : J * FREE + (j + 1) * MMN
                        ]
                        nc.tensor.matmul(
                            acc[:, j * MMN : (j + 1) * MMN],
                            lhsT,
                            rhs,
                            start=True,
                            stop=True,
                            tile_position=(b, 0),
                        )
                    # fused: colacc slice gets elementwise min; the DVE
                    # accumulator register carries the row minima.
                    cslice = colacc[:, J * FREE : (J + 1) * FREE]
                    if USE_CHAIN:
                        # J=0 reseeds the accumulator from BIG; J=1..MS-1
                        # chain the held register; only the last op reads it
                        # out. Explicit nosync deps pin DVE program order (the
                        # register state is invisible to the Tile scheduler).
                        inst = nc.vector._custom_dve(
                            CHAMFER_OP if J == 0 else CHAMFER_OP_CHAIN,
                            out=cslice,
                            accum_out=(
                                rowmins[:, i : i + 1] if J == MS - 1 else None
                            ),
                            in0=acc[:],
                            in1=cslice,
                            s0=BIG,
                        )
                        if prev_inst is not None:
                            bass._add_dep_helper(
                                inst.ins,
                                prev_inst.ins,
                                sync=False,
                                reason="chamfer accum register chain",
                            )
                        prev_inst = inst
                    else:
                        nc.vector._custom_dve(
                            CHAMFER_OP,
                            out=cslice,
                            accum_out=rowmins[:, i : i + 1],
                            in0=acc[:],
                            in1=cslice,
                            s0=rowmins[:, i : i + 1],
                        )

            # ---- finalize ----
            # rows: relu then sum -> [P, 1]
            rowrelu = cpool.tile([P, NB], F32)
            rowsum = cpool.tile([P, 1], F32)
            nc.vector.tensor_scalar_max(rowrelu[:], rowmins[:], 0.0)
            nc.vector.tensor_reduce(rowsum[:], rowrelu[:], axis=AX.X, op=OP.add)

            # cols: DVE cannot reduce across partitions. Transpose colacc in
            # 128x128 chunks on the PE (16 chunks per PSUM tile), then one 3D
            # reduce [P, 16, P] -> [P, 16] per tile.
            ident = cpool.tile([P, P], F32)
            make_identity(nc, ident[:])
            NCH = m // P
            CPT = FREE // P  # transposed chunks per psum tile
            colminT = cpool.tile([P, NCH], F32)
            for t in range(NCH // CPT):
                tp = ppool.tile([P, FREE], F32, tag="acc")
                for k in range(CPT):
                    kk = t * CPT + k
                    nc.tensor.transpose(
                        tp[:, k * P : (k + 1) * P],
                        colacc[:, kk * P : (kk + 1) * P],
                        ident[:],
                    )
                nc.vector.tensor_reduce(
                    colminT[:, t * CPT : (t + 1) * CPT],
                    tp[:].rearrange("p (k c) -> p k c", c=P),
                    axis=AX.X,
                    op=OP.min,
                )
            colrelu = cpool.tile([P, NCH], F32)
            colsum = cpool.tile([P, 1], F32)
            nc.vector.tensor_scalar_max(colrelu[:], colminT[:], 0.0)
            nc.vector.tensor_reduce(colsum[:], colrelu[:], axis=AX.X, op=OP.add)

            out_sb = cpool.tile([P, 2], F32)
            nc.vector.tensor_copy(out_sb[:, 0:1], rowsum[:])
            nc.vector.tensor_copy(out_sb[:, 1:2], colsum[:])
            nc.sync.dma_start(out_d[:], out_sb[:])

    nc.compile()
    return nc


def _augment(pred: np.ndarray, gt: np.ndarray):
    """pred [n,3], gt [m,3] f32 -> predT [5,n], gtT [5,m] f32."""
    n, m = pred.shape[0], gt.shape[0]
    predT = np.empty((5, n), np.float32)
    predT[0:3] = pred.T
    predT[3] = np.sum(pred.astype(np.float64) ** 2, axis=-1).astype(np.float32)
    predT[4] = 1.0
    gtT = np.empty((5, m), np.float32)
    gtT[0:3] = -2.0 * gt.T
    gtT[3] = 1.0
    gtT[4] = np.sum(gt.astype(np.float64) ** 2, axis=-1).astype(np.float32)
    return predT, gtT


_NC_CACHE = {}


def _get_nc(n, m, use_f32r=True):
    key = (n, m, use_f32r)
    if key not in _NC_CACHE:
        _NC_CACHE[key] = build_chamfer_nc(n, m, use_f32r)
    return _NC_CACHE[key]


def run_chamfer(pred: np.ndarray, gt: np.ndarray, use_f32r: bool = True, **kw):
    """pred [B,N,3], gt [B,M,3] -> (scalar, BassKernelResults)."""
    B, N, _ = pred.shape
    M = gt.shape[1]
    assert B <= 8
    nc = _get_nc(N, M, use_f32r)
    in_maps = []
    for b in range(B):
        predT, gtT = _augment(
            np.ascontiguousarray(pred[b], np.float32),
            np.ascontiguousarray(gt[b], np.float32),
        )
        in_maps.append({"predT": predT, "gtT": gtT})
    res = run_bass_kernel_spmd(nc, in_maps, core_ids=list(range(B)), **kw)
    vals = []
    for r in res.results:
        o = r["out"]
        p2q = float(o[:, 0].sum()) / N
        q2p = float(o[:, 1].sum()) / M
        vals.append(p2q + q2p)
    return np.float32(np.mean(vals)), res


def kernel(pred: np.ndarray, gt: np.ndarray) -> np.ndarray:
    val, _ = run_chamfer(np.asarray(pred), np.asarray(gt))
    return np.array(val, dtype=np.float32)


# revision 18
# speedup vs baseline: 1.9420x; 1.0073x over previous
"""Chamfer loss kernel for Trainium2 (8 NeuronCores, SPMD data-parallel over batch).

Problem: pred [8,8192,3], gt [8,8192,3] ->
    scalar = mean_b [ mean_n min_m d(b,n,m) + mean_m min_n d(b,n,m) ]
    d = max(||p-q||^2, 0)

Strategy (one batch element per core):
  - Augmented 5-dim matmul computes the full distance tile directly:
        P~_n = (p0,p1,p2, |p|^2, 1),  Q~_m = (-2q0,-2q1,-2q2, 1, |q|^2)
        dist[n,m] = P~_n . Q~_m
    float32r -> full-fp32-accurate result at ~1 cycle/row. The 4 matmuls of a
    [128 x 2048] PSUM supertile are packed into the 4 PE row groups
    (tile_position) so they run concurrently.
  - Flash-style: distances only ever exist in PSUM. A single custom DVE op
    (CHAMFER_MIN_MIN) per supertile does BOTH reductions in one pass:
        out      = min(tile, colacc)          (running column minima)
        accum    = min(seed, min_free(tile))  (running row minima)
    The stock Spec DSL can only fold the body (which would contaminate row
    minima with other row-blocks' values via colacc); we hand-edit the lowered
    uop so the accumulator ALU reads the raw Src0 delay lane instead of the
    body ALU output.
  - relu commutes with min -> applied after reduction.
  - Tail: col minima across partitions via PE transpose chunks + DVE reduce.
  - Per-core output: per-partition row/col relu'd min sums; host averages.
"""

import os
import sys

import numpy as np

for _p in ("/opt/trn_rl_repo",):
    if os.path.isdir(_p) and _p not in sys.path:
        sys.path.append(_p)

import concourse.bacc as bacc
import concourse.bass as bass
import concourse.mybir as mybir
import concourse.tile as tile
from concourse.bass_utils import run_bass_kernel_spmd
from concourse.masks import make_identity

F32 = mybir.dt.float32
F32R = mybir.dt.float32r
AX = mybir.AxisListType
OP = mybir.AluOpType

BIG = 3.0e38  # "+inf" seed for running minima


# ---------------------------------------------------------------------------
# Custom DVE op: out = min(in0, in1); accum_out = min(s0, min_free(in0))
# ---------------------------------------------------------------------------
def _register_chamfer_ops():
    import copy

    from concourse import dve_ops
    from concourse.dve_spec import Spec, Src0, Src1, minn, lower, AluOp, C0
    from concourse.dve_uop import AluInp, DveOpSpec

    if "CHAMFER_MIN_MIN" in dve_ops._SUB_OPCODE_FOR_NAME:
        found = {op.name: op for op in dve_ops.OPS}
        return found["CHAMFER_MIN_MIN"], found["CHAMFER_MIN_MIN_CHAIN"]

    def _ref(in0, in1, c0, c1, c2):
        out = np.minimum(in0, in1)
        accum = np.minimum(in0.min(axis=-1, keepdims=True), c0)
        return out, accum

    spec = Spec(
        body=minn(Src0, Src1),
        accum=AluOp.MIN,
        accum_init=C0,
        reference=_ref,
    )
    uops = lower(spec, ver="v3")
    # uops[0] = seed state (accumulator <- C0), uops[1] = steady state with
    # blk1 = MIN(src0=CURR_ALU_OUT, src1=PREV_ALU_OUT): src0 is the
    # accumulator feedback (blk1's own out flop), src1 the body value from
    # blk0. Repoint src1 to the raw Src0 riding blk0's delay lane 0 (same
    # pipeline tick as the body value) so the row-min accumulates the tile
    # alone, uncontaminated by colacc.
    st = uops[1].datapath_config[1]
    assert st.op == AluOp.MIN and st.alu_src0 == AluInp.CURR_ALU_OUT, (
        "dve_spec lowering layout changed; revisit CHAMFER_MIN_MIN uop edit"
    )
    assert st.alu_src1 == AluInp.PREV_ALU_OUT
    st.alu_src1 = AluInp.PREV_DELAY_0

    # Chain variant: uop0's accumulator stage holds the register value left by
    # the previous CHAMFER op (BYPASS of its own flop) instead of reseeding
    # from C0. Lets one row-block's 4 supertile ops share one accumulator with
    # a single readout on the last op.
    uops_chain = copy.deepcopy(uops)
    s0 = uops_chain[0].datapath_config[1]
    assert s0.op == AluOp.BYPASS and s0.alu_src0 == AluInp.PREV_DELAY_2
    s0.alu_src0 = AluInp.CURR_ALU_OUT
    s0.alu_src1 = AluInp.CURR_ALU_OUT

    base = max(dve_ops._SUB_OPCODE_FOR_NAME.values())
    assert base + 2 < 0x20

    class _HandEditedOp:
        def __init__(self, name, the_uops, opcode):
            self.name = name
            self.spec = spec
            self.subdim = False
            self.perf_en = {}
            self._opcode = opcode
            self._uops = the_uops
            self._compiled = {}

        def compile(self, ver):
            assert ver == "v3", "CHAMFER ops are TRN2-only"
            if ver not in self._compiled:
                self._compiled[ver] = DveOpSpec(
                    name=self.name, opcode=self._opcode, uops=self._uops, rd1_en=True
                )
            return self._compiled[ver]

    op_seed = _HandEditedOp("CHAMFER_MIN_MIN", uops, base + 1)
    op_chain = _HandEditedOp("CHAMFER_MIN_MIN_CHAIN", uops_chain, base + 2)
    for op in (op_seed, op_chain):
        dve_ops.OPS.append(op)
        dve_ops._SUB_OPCODE_FOR_NAME[op.name] = op._opcode
    return op_seed, op_chain


CHAMFER_OP, CHAMFER_OP_CHAIN = _register_chamfer_ops()
USE_CHAIN = True


def build_chamfer_nc(n: int, m: int, use_f32r: bool = True):
    """Build the per-core chamfer kernel graph.

    Inputs (per core): predT [5, n] f32 (augmented, transposed),
                       gtT   [5, m] f32 (augmented, transposed).
    Output: out [128, 2] f32.
        out[:, 0]  = per-partition sums over n-blocks of relu(row minima)
        out[:, 1]  = per-partition sums of relu(col minima)
    """
    P = 128
    FREE = 2048  # m supertile (4 PSUM banks)
    MMN = 512  # free dim per matmul (1 PSUM bank, fp32)
    assert n % P == 0 and m % FREE == 0
    NB = n // P
    MS = m // FREE
    NMM = FREE // MMN
    assert NMM == 4

    nc = bacc.Bacc("TRN2", target_bir_lowering=False, debug=False)
    mm_dt = F32R if use_f32r else F32
    predT_d = nc.dram_tensor("predT", [5, n], mm_dt, kind="ExternalInput")
    gtT_d = nc.dram_tensor("gtT", [5, m], mm_dt, kind="ExternalInput")
    out_d = nc.dram_tensor("out", [P, 2], F32, kind="ExternalOutput")

    with tile.TileContext(nc) as tc:
        with (
            tc.tile_pool(name="const", bufs=1) as cpool,
            tc.tile_pool(name="psum", bufs=2, space=bass.MemorySpace.PSUM) as ppool,
        ):
            # pred/gt replicated at the 4 PE row-groups (base partitions
            # 0/32/64/96) so the 4 matmuls of a supertile run concurrently
            # via tile_position row packing.
            predT = cpool.tile([96 + 5, n], mm_dt)
            gtT = cpool.tile([96 + 5, m], mm_dt)
            # spread the 8 replica loads across 3 DGE queues
            engines = [nc.sync, nc.gpsimd, nc.scalar]
            for g in range(NMM):
                engines[g % 3].dma_start(predT[32 * g : 32 * g + 5, :], predT_d[:])
                engines[(g + 1) % 3].dma_start(gtT[32 * g : 32 * g + 5, :], gtT_d[:])

            colacc = cpool.tile([P, m], F32)
            nc.vector.memset(colacc[:], BIG)
            rowmins = cpool.tile([P, NB], F32)
            nc.vector.memset(rowmins[:], BIG)

            prev_inst = None
            for i in range(NB):
                for J in range(MS):
                    acc = ppool.tile([P, FREE], F32, tag="acc")
                    for j in range(NMM):
                        b = 32 * j
                        lhsT = predT[b : b + 5, i * P : (i + 1) * P]
                        rhs = gtT[
                            b : b + 5, J * FREE + j * MMN : J * FREE + (j + 1) * MMN
                        ]
                        nc.tensor.matmul(
                            acc[:, j * MMN : (j + 1) * MMN],
                            lhsT,
                            rhs,
                            start=True,
                            stop=True,
                            tile_position=(b, 0),
                        )
                    # fused: colacc slice gets elementwise min; the DVE
                    # accumulator register carries the row minima.
                    cslice = colacc[:, J * FREE : (J + 1) * FREE]
                    if USE_CHAIN:
                        # J=0 reseeds the accumulator from BIG; J=1..MS-1
                        # chain the held register; only the last op reads it
                        # out. Explicit nosync deps pin DVE program order (the
                        # register state is invisible to the Tile scheduler).
                        inst = nc.vector._custom_dve(
                            CHAMFER_OP if J == 0 else CHAMFER_OP_CHAIN,
                            out=cslice,
                            accum_out=(
                                rowmins[:, i : i + 1] if J == MS - 1 else None
                            ),
                            in0=acc[:],
                            in1=cslice,
                            s0=BIG,
                        )
                        if prev_inst is not None:
                            bass._add_dep_helper(
                                inst.ins,
                                prev_inst.ins,
                                sync=False,
                                reason="chamfer accum register chain",
                            )
                        prev_inst = inst
                    else:
                        nc.vector._custom_dve(
                            CHAMFER_OP,
                            out=cslice,
                            accum_out=rowmins[:, i : i + 1],
                            in0=acc[:],
                            in1=cslice,
                            s0=rowmins[:, i : i + 1],
                        )

            # ---- finalize ----
            # rows: relu then sum -> [P, 1]
            rowrelu = cpool.tile([P, NB], F32)
            rowsum = cpool.tile([P, 1], F32)
            nc.vector.tensor_scalar_max(rowrelu[:], rowmins[:], 0.0)
            nc.vector.tensor_reduce(rowsum[:], rowrelu[:], axis=AX.X, op=OP.add)

            # cols: DVE cannot reduce across partitions. Transpose colacc in
            # 128x128 chunks on the PE (16 chunks per PSUM tile), then one 3D
            # reduce [P, 16, P] -> [P, 16] per tile.
            ident = cpool.tile([P, P], F32)
            make_identity(nc, ident[:])
            NCH = m // P
            CPT = FREE // P  # transposed chunks per psum tile
            colminT = cpool.tile([P, NCH], F32)
            for t in range(NCH // CPT):
                tp = ppool.tile([P, FREE], F32, tag="acc")
                for k in range(CPT):
                    kk = t * CPT + k
                    nc.tensor.transpose(
                        tp[:, k * P : (k + 1) * P],
                        colacc[:, kk * P : (kk + 1) * P],
                        ident[:],
                    )
                nc.vector.tensor_reduce(
                    colminT[:, t * CPT : (t + 1) * CPT],
                    tp[:].rearrange("p (k c) -> p k c", c=P),
                    axis=AX.X,
                    op=OP.min,
                )
            colrelu = cpool.tile([P, NCH], F32)
            colsum = cpool.tile([P, 1], F32)
            nc.vector.tensor_scalar_max(colrelu[:], colminT[:], 0.0)
            nc.vector.tensor_reduce(colsum[:], colrelu[:], axis=AX.X, op=OP.add)

            out_sb = cpool.tile([P, 2], F32)
            nc.vector.tensor_copy(out_sb[:, 0:1], rowsum[:])
            nc.vector.tensor_copy(out_sb[:, 1:2], colsum[:])
            nc.sync.dma_start(out_d[:], out_sb[:])

    nc.compile()
    return nc


def _augment(pred: np.ndarray, gt: np.ndarray):
    """pred [n,3], gt [m,3] f32 -> predT [5,n], gtT [5,m] f32."""
    n, m = pred.shape[0], gt.shape[0]
    predT = np.empty((5, n), np.float32)
    predT[0:3] = pred.T
    predT[3] = np.sum(pred.astype(np.float64) ** 2, axis=-1).astype(np.float32)
    predT[4] = 1.0
    gtT = np.empty((5, m), np.float32)
    gtT[0:3] = -2.0 * gt.T
    gtT[3] = 1.0
    gtT[4] = np.sum(gt.astype(np.float64) ** 2, axis=-1).astype(np.float32)
    return predT, gtT


_NC_CACHE = {}


def _get_nc(n, m, use_f32r=True):
    key = (n, m, use_f32r)
    if key not in _NC_CACHE:
        _NC_CACHE[key] = build_chamfer_nc(n, m, use_f32r)
    return _NC_CACHE[key]


def run_chamfer(pred: np.ndarray, gt: np.ndarray, use_f32r: bool = True, **kw):
    """pred [B,N,3], gt [B,M,3] -> (scalar, BassKernelResults)."""
    B, N, _ = pred.shape
    M = gt.shape[1]
    assert B <= 8
    nc = _get_nc(N, M, use_f32r)
    in_maps = []
    for b in range(B):
        predT, gtT = _augment(
            np.ascontiguousarray(pred[b], np.float32),
            np.ascontiguousarray(gt[b], np.float32),
        )
        in_maps.append({"predT": predT, "gtT": gtT})
    res = run_bass_kernel_spmd(nc, in_maps, core_ids=list(range(B)), **kw)
    vals = []
    for r in res.results:
        o = r["out"]
        p2q = float(o[:, 0].sum()) / N
        q2p = float(o[:, 1].sum()) / M
        vals.append(p2q + q2p)
    return np.float32(np.mean(vals)), res


def kernel(pred: np.ndarray, gt: np.ndarray) -> np.ndarray:
    val, _ = run_chamfer(np.asarray(pred), np.asarray(gt))
    return np.array(val, dtype=np.float32)
